# revision 31
# baseline (speedup 1.0000x reference)
"""MultiHeadLatentAttention on 8 Trainium2 NeuronCores (Bass/Tile, SPMD).

Sharding (tensor parallel over heads, per the hint, plus refinements):
  - 16 heads / 8 cores = 2 heads per core: q_proj + kv_b_proj output dims and
    o_proj input dim sharded by head.
  - kv_a_proj + rms-norm are token-sharded (512 tokens/core) with an
    AllGather of the normalized latent (bf16, 0.5 MB/core).
  - AllToAll of attention outputs (bf16, 1 MB/batch) token-shards o_proj:
    each core computes the full o_proj for its 512 tokens.

v2 changes vs baseline:
  - bf16 operands for every matmul (rel err ~5e-3 vs the 2e-2 gate); psum
    and softmax stay fp32. Halves HBM traffic and SBUF footprint.
  - softmax denominator: exp tiles are accumulated on the (otherwise idle)
    Pool engine; one [1,512] ones-matmul + one broadcast matmul per q-chunk
    replaces 16 M=1 matmuls (-123k PE cycles/core).
  - o_proj runs batch-major with o_w resident in SBUF: batch-0 o_proj
    matmuls are interleaved into the tail of batch-1's ACT-paced attention
    (late enough that batch-0's AllToAll has landed), the rest plus all of
    batch-1 run as a deep-buffered tail that hides batch-1's AllToAll.
  - psum->sbuf copies placed on the Scalar engine in phases where it idles.
"""

import math
from contextlib import ExitStack

import numpy as np

B, S = 2, 2048
T = B * S                     # 4096 flattened tokens
HID = 2048
H, D = 16, 128
RANK, ROPE = 512, 64
MAX_POS, ORIG_POS = 131072, 8192
BASE = 500000.0
BETA_FAST, BETA_SLOW = 32.0, 1.0
EPS = 1e-6
NCORES = 8
HPC = H // NCORES             # 2 heads per core
TPC = T // NCORES             # 512 tokens per core (kv_a shard)
SPC = S // NCORES             # 256 tokens per (core, batch) after AllToAll
NH = HID // 128               # 16 hid chunks
NR = RANK // 128              # 4 rank chunks
NKT = S // 128                # 16 k-chunks per batch

_CACHE: dict = {}


def _yarn_cos_sin():
    """cos/sin tables matching reference.py's yarn_cos_sin (mscale folded)."""
    scaling = MAX_POS / ORIG_POS
    pos_freqs = BASE ** (np.arange(0, ROPE, 2, dtype=np.float64) / ROPE)
    extrap = 1.0 / pos_freqs
    interp = 1.0 / (scaling * pos_freqs)
    low = max(math.floor(ROPE * math.log(ORIG_POS / (BETA_FAST * 2 * math.pi))
                         / (2 * math.log(BASE))), 0)
    high = min(math.ceil(ROPE * math.log(ORIG_POS / (BETA_SLOW * 2 * math.pi))
                         / (2 * math.log(BASE))), ROPE - 1)
    i = np.arange(ROPE // 2, dtype=np.float64)
    smooth = np.clip((i - low) / max(high - low, 1), 0.0, 1.0)
    inv_freq = ((1.0 - smooth) * interp + smooth * extrap).astype(np.float32)
    pos = np.arange(S, dtype=np.float32)
    freqs = pos[:, None] * inv_freq[None, :]              # [S, 32]
    emb = np.concatenate([freqs, freqs], axis=-1)         # [S, 64]
    mscale = 0.1 * math.log(scaling) + 1.0
    cos = (np.cos(emb) * mscale).astype(np.float32)
    sin = (np.sin(emb) * mscale).astype(np.float32)
    return cos.T.copy(), sin.T.copy()                     # [64, S] each


OPT = {
    "interleave_o": True,    # o_proj(b0) interleaved into P5(b1)
    "interleave_from": 4,    # first P5(b1) iter (0-7) to interleave at
    "dma_transpose": False,  # v via DMA XBAR transpose (else PE transpose)
    "coltile_den": True,     # col-tiled denominator matmuls
}


def build_nc(passes=1, sim_mode=False):
    """Build + compile the (single, SPMD) Bass program for all 8 cores."""
    import concourse.tile as tile
    import concourse.mybir as mybir
    from concourse import bacc

    F32 = mybir.dt.float32
    F32R = mybir.dt.float32r
    BF = mybir.dt.bfloat16
    AF = mybir.ActivationFunctionType
    RG = [list(range(NCORES))]

    nc = bacc.Bacc("TRN2", target_bir_lowering=False, debug=False,
                   num_devices=1 if sim_mode else NCORES)

    # ---- kernel I/O (all matmul operands pre-cast to bf16 on host) ----
    hsT_in = nc.dram_tensor("hsT", [HID, T], BF, kind="ExternalInput").ap()
    hsmy_in = nc.dram_tensor("hsmy", [HID, TPC], BF, kind="ExternalInput").ap()
    qwT_in = nc.dram_tensor("qwT", [HID, HPC * D], BF, kind="ExternalInput").ap()
    kvaT_in = nc.dram_tensor("kvaT", [HID, RANK], BF, kind="ExternalInput").ap()
    kvbT_in = nc.dram_tensor("kvbT", [RANK, HPC * 2 * D], BF, kind="ExternalInput").ap()
    owt_in = nc.dram_tensor("owt", [16, 128, HID], BF, kind="ExternalInput").ap()
    cos_in = nc.dram_tensor("cos", [ROPE, S], BF, kind="ExternalInput").ap()
    sinsh_in = nc.dram_tensor("sinsh", [ROPE, S], BF, kind="ExternalInput").ap()
    ones_in = nc.dram_tensor("ones", [128, 128], F32R, kind="ExternalInput").ap()
    onesbf_in = nc.dram_tensor("onesbf", [128, 32], BF, kind="ExternalInput").ap()
    ones32_in = nc.dram_tensor("ones32", [32, 128], F32R, kind="ExternalInput").ap()
    identbf_in = nc.dram_tensor("identbf", [128, 128], BF, kind="ExternalInput").ap()
    outTs = [nc.dram_tensor(f"outT{p}" if p else "outT", [HID, 2 * SPC], F32,
                            kind="ExternalOutput").ap() for p in range(passes)]

    with tile.TileContext(nc) as tc, ExitStack() as ctx0:
        const = ctx0.enter_context(tc.tile_pool(name="const", bufs=1))
        dram = ctx0.enter_context(tc.tile_pool(name="dram", bufs=1, space="DRAM"))

        ones = const.tile([128, 128], F32R)
        ones_bf = const.tile([128, 32], BF)
        ones32 = const.tile([32, 128], F32R)
        identbf = const.tile([128, 128], BF)
        cosb = const.tile([ROPE, S], BF)
        sinsh = const.tile([ROPE, S], BF)
        eps_t = const.tile([1, 1], F32)
        nc.sync.dma_start(ones[:], ones_in[:])
        nc.sync.dma_start(ones_bf[:], onesbf_in[:])
        nc.sync.dma_start(ones32[:], ones32_in[:])
        nc.sync.dma_start(identbf[:], identbf_in[:])
        nc.sync.dma_start(cosb[:], cos_in[:])
        nc.sync.dma_start(sinsh[:], sinsh_in[:])
        nc.vector.memset(eps_t[:], EPS)

        # o_proj weights resident in SBUF for the whole kernel (8.4 MB bf16);
        # the load is issued after P2's critical DMAs, not here.
        owt_sb = const.tile([128, 16 * HID], BF, name="owt_sb")

        for p_ in range(passes):
            # collective bounce buffers
            ag_in = [dram.tile([RANK // 2, TPC], BF, name=f"agin{p_}{h}")
                     for h in range(2)]
            ag_out = [dram.tile([NCORES, RANK // 2, TPC], BF,
                                addr_space="Local" if sim_mode else "Shared",
                                name=f"agout{p_}{h}") for h in range(2)]
            a2a_in = [dram.tile([NCORES, HPC * D, SPC], BF, name=f"a2ain{p_}{b}")
                      for b in range(B)]
            a2a_out = [dram.tile([NCORES, HPC * D, SPC], BF, name=f"a2aout{p_}{b}")
                       for b in range(B)]

            ctx_pass = ExitStack()
            afp = ctx_pass.enter_context(tc.tile_pool(name=f"afp_{p_}", bufs=1))
            # attention outputs post-A2A, layout [k16][batch][SPC] so one
            # o_proj matmul can move both batches (N=512)
            af = afp.tile([128, NH * 2 * SPC], BF, name=f"af{p_}")

            with ExitStack() as ctx_big:
                big = ctx_big.enter_context(tc.tile_pool(name=f"big_{p_}", bufs=1))
                rope_pool = ctx_big.enter_context(
                    tc.tile_pool(name=f"rope_{p_}", bufs=1))

                def rope_block(X):
                    tmp = rope_pool.tile([ROPE, S], BF, tag="rtmp", bufs=1,
                                         name="rtmp")
                    m2 = rope_pool.tile([ROPE, S], BF, tag="rm2", bufs=1,
                                        name="rm2")
                    nc.vector.tensor_mul(tmp[:], X[0:ROPE], cosb[:])
                    nc.vector.tensor_mul(m2[0:32], X[32:64], sinsh[32:64])
                    nc.vector.tensor_mul(m2[32:64], X[0:32], sinsh[0:32])
                    nc.vector.tensor_add(X[0:ROPE], tmp[:], m2[:])

                # per (head j, batch b) tiles, [128, S] bf16 each
                qT = [[big.tile([128, S], BF, name=f"qT{p_}{j}{b}") for b in range(B)]
                      for j in range(HPC)]
                kT = [[big.tile([128, S], BF, name=f"kT{p_}{j}{b}") for b in range(B)]
                      for j in range(HPC)]
                vnat = [[big.tile([128, S], BF, name=f"vn{p_}{j}{b}") for b in range(B)]
                        for j in range(HPC)]

                # ---------- P1: kv_a on my 512-token shard + rms norm + AllGather
                with ExitStack() as c1:
                    p1 = c1.enter_context(tc.tile_pool(name=f"p1_{p_}", bufs=1))
                    p1ps = c1.enter_context(tc.tile_pool(name=f"p1ps_{p_}", bufs=1, space="PSUM"))
                    ps_lat = [p1ps.tile([128, TPC], F32, name=f"pslat{p_}{m}", tag=f"lat{m}")
                              for m in range(NR)]
                    for k in range(NH):
                        kva_t = p1.tile([128, RANK], BF, tag="kvat", bufs=3)
                        nc.sync.dma_start(kva_t[:], kvaT_in[k * 128:(k + 1) * 128, :])
                        ht = p1.tile([128, TPC], BF, tag="hsmy", bufs=4)
                        nc.sync.dma_start(ht[:], hsmy_in[k * 128:(k + 1) * 128, :])
                        for m in range(NR):
                            nc.tensor.matmul(
                                ps_lat[m][:],
                                kva_t[:, m * 128:(m + 1) * 128],
                                ht[:], start=(k == 0), stop=(k == NH - 1))
                    # rms norm over rank (partition axis, 4 chunks)
                    lat_sb = p1.tile([128, NR * TPC], F32R)
                    ps_var = p1ps.tile([1, TPC], F32, tag="var")
                    for m in range(NR):
                        nc.scalar.copy(lat_sb[:, m * TPC:(m + 1) * TPC], ps_lat[m][:])
                    sq = [p1.tile([128, TPC], F32R, name=f"sq{p_}{m}", tag="sq", bufs=2)
                          for m in range(NR)]
                    for m in range(NR):
                        nc.vector.tensor_mul(sq[m][:], lat_sb[:, m * TPC:(m + 1) * TPC],
                                             lat_sb[:, m * TPC:(m + 1) * TPC])
                        nc.tensor.matmul(ps_var[:], ones[:, 0:1], sq[m][:],
                                         start=(m == 0), stop=(m == NR - 1))
                    std = p1.tile([1, TPC], F32R, tag="std")
                    nc.scalar.activation(std[:], ps_var[:], AF.Sqrt,
                                         bias=eps_t[:], scale=1.0 / RANK)
                    istd = p1.tile([1, TPC], F32R, tag="istd")
                    with nc.allow_low_precision(reason="f32r holds full f32 bits"):
                        nc.vector.reciprocal(istd[:], std[:])
                    ps_bc = p1ps.tile([128, TPC], F32, tag="bc")
                    nc.tensor.matmul(ps_bc[:], ones[0:1, :], istd[:],
                                     start=True, stop=True)
                    latn = p1.tile([128, NR * TPC], BF)
                    for h in range(2):
                        for m2 in range(2):
                            m = 2 * h + m2
                            nc.vector.tensor_mul(latn[:, m * TPC:(m + 1) * TPC],
                                                 lat_sb[:, m * TPC:(m + 1) * TPC],
                                                 ps_bc[:])
                            nc.sync.dma_start(ag_in[h][m2 * 128:(m2 + 1) * 128, :],
                                              latn[:, m * TPC:(m + 1) * TPC])
                        if sim_mode:
                            for s8 in range(NCORES):
                                nc.sync.dma_start(ag_out[h][s8], ag_in[h][:])
                        else:
                            nc.gpsimd.collective_compute(
                                "AllGather", mybir.AluOpType.bypass,
                                replica_groups=RG,
                                ins=[ag_in[h].opt()], outs=[ag_out[h].opt()])

                # ---------- P2: q_proj for my 2 heads, one 2048-token group per batch
                with ExitStack() as c2:
                    p2 = c2.enter_context(tc.tile_pool(name=f"p2_{p_}", bufs=1))
                    p2ps = c2.enter_context(tc.tile_pool(name=f"p2ps_{p_}", bufs=1, space="PSUM"))
                    for g in range(B):            # 2048-token groups (= batch)
                        psq = [[p2ps.tile([128, 512], F32, name=f"psq{p_}{g}{m}{t4}",
                                          tag="psq", bufs=8)
                                for t4 in range(4)] for m in range(HPC)]
                        for k in range(NH):
                            qw_t = p2.tile([128, HPC * D], BF, tag="qwt", bufs=3)
                            nc.sync.dma_start(qw_t[:],
                                              qwT_in[k * 128:(k + 1) * 128, :])
                            ht = p2.tile([128, S], BF, tag="hsq", bufs=4)
                            nc.sync.dma_start(
                                ht[:], hsT_in[k * 128:(k + 1) * 128,
                                              g * S:(g + 1) * S])
                            for m in range(HPC):
                                for t4 in range(4):
                                    nc.tensor.matmul(
                                        psq[m][t4][:],
                                        qw_t[:, m * 128:(m + 1) * 128],
                                        ht[:, t4 * 512:(t4 + 1) * 512],
                                        start=(k == 0), stop=(k == NH - 1))
                        for m in range(HPC):
                            for t4 in range(4):
                                nc.scalar.copy(qT[m][g][:, t4 * 512:(t4 + 1) * 512],
                                               psq[m][t4][:])
                        for j in range(HPC):
                            rope_block(qT[j][g])

                if p_ == 0:
                    # o_proj weights: issue after P1/P2's critical loads, well
                    # before the first o_proj matmul (mid-P5).
                    nc.sync.dma_start(
                        owt_sb[:].rearrange("p (c m) -> p c m", c=16),
                        owt_in.rearrange("c p m -> p c m"))

                # ---------- P3: kv_b for my 2 heads (k direct, v via DMA transpose)
                with ExitStack() as c3:
                    p3 = c3.enter_context(tc.tile_pool(name=f"p3_{p_}", bufs=1))
                    p3ps = c3.enter_context(tc.tile_pool(name=f"p3ps_{p_}", bufs=1, space="PSUM"))
                    kvbT_sb = p3.tile([128, NR * HPC * 2 * D], BF)
                    nc.sync.dma_start(
                        kvbT_sb[:].rearrange("p (r m) -> p r m", r=NR),
                        kvbT_in.rearrange("(r p) m -> p r m", p=128))
                    for tc8 in range(NCORES):     # 512-token chunks (AG layout)
                        b, loc = tc8 // 4, (tc8 % 4) * 512
                        lt = [p3.tile([128, 2 * 512], BF, tag=f"lt{h}", bufs=2,
                                      name=f"lth{h}") for h in range(2)]
                        for h in range(2):
                            nc.sync.dma_start(
                                lt[h][:].rearrange("p (r t) -> p r t", r=2),
                                ag_out[h][tc8].rearrange("(r p) t -> p r t", p=128))
                        for m in range(2 * HPC):  # k0,v0,k1,v1
                            j, is_v = m // 2, m % 2
                            ps = p3ps.tile([128, 512], F32, tag="kv", bufs=4)
                            for r in range(NR):
                                nc.tensor.matmul(
                                    ps[:],
                                    kvbT_sb[:, r * HPC * 2 * D + m * 128:
                                            r * HPC * 2 * D + (m + 1) * 128],
                                    lt[r // 2][:, (r % 2) * 512:(r % 2 + 1) * 512],
                                    start=(r == 0), stop=(r == NR - 1))
                            if not is_v:
                                nc.scalar.copy(kT[j][b][:, loc:loc + 512], ps[:])
                            elif OPT["dma_transpose"]:
                                vt = p3.tile([128, 512], BF, tag="vt", bufs=3)
                                nc.scalar.copy(vt[:], ps[:])
                                for q4 in range(4):
                                    nc.sync.dma_start_transpose(
                                        vnat[j][b][:, loc + q4 * 128: loc + (q4 + 1) * 128],
                                        vt[:, q4 * 128:(q4 + 1) * 128])
                            else:
                                vt = p3.tile([128, 512], BF, tag="vt", bufs=3)
                                nc.scalar.copy(vt[:], ps[:])
                                for q4 in range(4):
                                    pst = p3ps.tile([128, 128], BF, tag="pst", bufs=2)
                                    nc.tensor.transpose(
                                        pst[:], vt[:, q4 * 128:(q4 + 1) * 128],
                                        identbf[:])
                                    nc.vector.tensor_copy(
                                        vnat[j][b][:, loc + q4 * 128: loc + (q4 + 1) * 128],
                                        pst[:])
                        if tc8 % 4 == 3:
                            for j in range(HPC):
                                rope_block(kT[j][b])

                # ---------- P5: attention per (batch, head), scoresT layout
                # (+ o_proj: batch 0 interleaved, batch 1 as the tail)
                o_done: set = set()

                def make_o_proj_block(ps_pool, sb_pool, ps_bufs):
                    def o_proj_block(b, om):
                        """One o_proj chunk: rows [om*128,(om+1)*128).
                        b in {0,1}: that batch's 256 tokens (N=256).
                        b is None: both batches in one sweep (N=512)."""
                        if b is None:
                            o_done.update({(0, om), (1, om)})
                            cols, n = slice(0, 2 * SPC), 2 * SPC
                        else:
                            o_done.add((b, om))
                            cols = slice(b * SPC, (b + 1) * SPC)
                            n = SPC
                        tag = "oc" if b is None else "o"
                        ps_o = ps_pool.tile([128, n], F32, tag=tag,
                                            bufs=ps_bufs, name="pso" + tag)
                        c0 = 0 if b is None else b * SPC
                        for k16 in range(NH):
                            base = k16 * 2 * SPC + c0
                            nc.tensor.matmul(
                                ps_o[:],
                                owt_sb[:, om * HID + k16 * 128:
                                       om * HID + (k16 + 1) * 128],
                                af[:, base:base + n],
                                start=(k16 == 0), stop=(k16 == NH - 1))
                        o_sb = sb_pool.tile([128, n], F32, tag="osb" + tag,
                                            bufs=3, name="osb")
                        nc.scalar.copy(o_sb[:], ps_o[:])
                        nc.sync.dma_start(
                            outTs[p_][om * 128:(om + 1) * 128, cols],
                            o_sb[:])
                    return o_proj_block

                with ExitStack() as c5:
                    p5 = c5.enter_context(tc.tile_pool(name=f"p5_{p_}", bufs=1))
                    p57ps = c5.enter_context(
                        tc.tile_pool(name=f"p57ps_{p_}", bufs=1, space="PSUM"))
                    o_proj_block = make_o_proj_block(p57ps, p5, 1)

                    for b in range(B):
                        for j in range(HPC):
                            qt, kt, vn = qT[j][b], kT[j][b], vnat[j][b]
                            for qc in range(4):
                                qs = qt[:, qc * 512:(qc + 1) * 512]
                                ps_av = p57ps.tile([128, 512], F32, tag="av", bufs=2)
                                ps_den = p57ps.tile([128, 512], F32, tag="den", bufs=1)
                                es_of = {}

                                def den_mms(k16_base):
                                    # 4 col-tiled M=32 ones-matmuls, one per
                                    # 32-col PE group -> run ~4x concurrent.
                                    # Group g accumulates k16 = g, g+4, g+8, g+12
                                    # into ps_den[32g:32g+32].
                                    for g in range(4):
                                        k16 = k16_base + g
                                        nc.tensor.matmul(
                                            ps_den[32 * g:32 * (g + 1), :],
                                            ones_bf[:, 0:32], es_of.pop(k16),
                                            start=(k16 < 4), stop=(k16 >= 12),
                                            tile_position=(0, 32 * g))

                                for kp in range(NKT // 2):
                                    ps_s = p57ps.tile([128, 1024], F32, tag="s", bufs=2)
                                    e = p5.tile([128, 1024], BF, tag="e", bufs=4)
                                    for h2 in range(2):
                                        k16 = 2 * kp + h2
                                        nc.tensor.matmul(
                                            ps_s[:, h2 * 512:(h2 + 1) * 512],
                                            kt[:, k16 * 128:(k16 + 1) * 128], qs,
                                            start=True, stop=True)
                                    nc.scalar.activation(e[:], ps_s[:], AF.Exp)
                                    for h2 in range(2):
                                        k16 = 2 * kp + h2
                                        es = e[:, h2 * 512:(h2 + 1) * 512]
                                        es_of[k16] = es
                                        nc.tensor.matmul(
                                            ps_av[:], vn[:, k16 * 128:(k16 + 1) * 128], es,
                                            start=(k16 == 0), stop=(k16 == NKT - 1))
                                        if not OPT["coltile_den"]:
                                            nc.tensor.matmul(
                                                ps_den[0:1, :], ones_bf[:, 0:1],
                                                es_of.pop(k16),
                                                start=(k16 == 0),
                                                stop=(k16 == NKT - 1))
                                    if OPT["coltile_den"] and kp in (2, 4, 6):
                                        den_mms(2 * kp - 4)
                                if OPT["coltile_den"]:
                                    den_mms(12)
                                    # sum the 4 bands, broadcast, normalize
                                    bsum = p5.tile([32, 512], F32R, tag="bsum",
                                                   bufs=3)
                                    nc.vector.tensor_copy(bsum[:], ps_den[0:32, :])
                                    nc.vector.tensor_add(bsum[:], bsum[:],
                                                         ps_den[32:64, :])
                                    nc.vector.tensor_add(bsum[:], bsum[:],
                                                         ps_den[64:96, :])
                                    nc.vector.tensor_add(bsum[:], bsum[:],
                                                         ps_den[96:128, :])
                                    nc.tensor.matmul(ps_den[:], ones32[:], bsum[:],
                                                     start=True, stop=True)
                                else:
                                    den_sb = p5.tile([1, 512], F32R, tag="densb",
                                                     bufs=3)
                                    nc.vector.tensor_copy(den_sb[:],
                                                          ps_den[0:1, :])
                                    nc.tensor.matmul(ps_den[:], ones[0:1, :],
                                                     den_sb[:],
                                                     start=True, stop=True)
                                rec = p5.tile([128, 512], F32, tag="rec", bufs=3)
                                nc.vector.reciprocal(rec[:], ps_den[:])
                                ao_t = p5.tile([128, 512], BF, tag="aot", bufs=4)
                                nc.vector.tensor_mul(ao_t[:], ps_av[:], rec[:])
                                for h2a in range(2):
                                    s8 = 2 * qc + h2a
                                    nc.sync.dma_start(
                                        a2a_in[b][s8, j * D:(j + 1) * D, :],
                                        ao_t[:, h2a * SPC:(h2a + 1) * SPC])
                                # interleave batch-0 o_proj into batch-1's
                                # ACT-paced attention to fill PE slack
                                if b == 1 and OPT["interleave_o"]:
                                    it = j * 4 + qc
                                    if it >= OPT["interleave_from"]:
                                        o0 = 2 * (it - OPT["interleave_from"])
                                        o_proj_block(0, o0)
                                        o_proj_block(0, o0 + 1)
                        # AllToAll for this batch as soon as both heads are done
                        if sim_mode:
                            nc.sync.dma_start(a2a_out[b][:], a2a_in[b][:])
                        else:
                            nc.gpsimd.collective_compute(
                                "AllToAll", mybir.AluOpType.bypass, replica_groups=RG,
                                ins=[a2a_in[b].opt()], outs=[a2a_out[b].opt()])
                        for k16 in range(NH):
                            i, halfk = k16 // 2, k16 % 2
                            col = k16 * 2 * SPC + b * SPC
                            nc.sync.dma_start(
                                af[:, col:col + SPC],
                                a2a_out[b][i, halfk * 128:(halfk + 1) * 128, :])

            # ---------- P7 tail: remaining o_proj chunks (own psum pool: the
            # attention banks are free now, so deep buffering, no stalls)
            with ExitStack() as c7:
                p7 = c7.enter_context(tc.tile_pool(name=f"p7_{p_}", bufs=1))
                p7ps = c7.enter_context(
                    tc.tile_pool(name=f"p7ps_{p_}", bufs=1, space="PSUM"))
                o_tail = make_o_proj_block(p7ps, p7, 2)
                for om in range(NH):
                    if (0, om) not in o_done and (1, om) not in o_done:
                        o_tail(None, om)       # both batches, N=512
                for b in range(B):
                    for om in range(NH):
                        if (b, om) not in o_done:
                            o_tail(b, om)
            ctx_pass.close()

    nc.compile()
    return nc


def build_in_maps(hidden_states, q_w, kv_a_w, kv_b_w, o_w, kv_norm_w):
    import ml_dtypes
    bf16 = ml_dtypes.bfloat16

    hs = np.ascontiguousarray(np.asarray(hidden_states, dtype=np.float32))
    q_w = np.asarray(q_w, dtype=np.float32)
    kv_a_w = np.asarray(kv_a_w, dtype=np.float32)
    kv_b_w = np.asarray(kv_b_w, dtype=np.float32)
    o_w = np.asarray(o_w, dtype=np.float32)
    kv_norm_w = np.asarray(kv_norm_w, dtype=np.float32)

    hsT = np.ascontiguousarray(hs.reshape(T, HID).T).astype(bf16)     # [HID, T]
    kvaT = np.ascontiguousarray(kv_a_w[ROPE:, :].T).astype(bf16)      # [HID, RANK]
    scale = D ** -0.5
    cosT, sinT = _yarn_cos_sin()
    sinsh = np.concatenate([sinT[32:64], -sinT[0:32]], axis=0)
    ones = np.ones((128, 128), dtype=np.float32)
    # owt[om, p, k*128+m] = o_w[om*128+m, k*128+p]
    owt = np.ascontiguousarray(
        o_w.reshape(16, 128, 16, 128).transpose(0, 3, 2, 1).reshape(16, 128, HID)
    ).astype(bf16)

    kvb = (kv_b_w * kv_norm_w[None, :]).reshape(H, 2, D, RANK)

    in_maps = []
    for c in range(NCORES):
        qwT = np.ascontiguousarray(
            (q_w[c * HPC * D:(c + 1) * HPC * D] * scale).T).astype(bf16)  # [HID, 256]
        # kvbT rows order per core: k0,v0,k1,v1 each 128 wide
        blk = kvb[c * HPC:(c + 1) * HPC]                               # [2,2,128,RANK]
        kvbT = np.ascontiguousarray(
            blk.reshape(HPC * 2 * D, RANK).T).astype(bf16)             # [RANK, 512]
        hsmy = np.ascontiguousarray(hsT[:, c * TPC:(c + 1) * TPC])
        in_maps.append({
            "hsT": hsT, "hsmy": hsmy, "qwT": qwT, "kvaT": kvaT,
            "kvbT": kvbT, "owt": owt,
            "cos": cosT.astype(bf16), "sinsh": sinsh.astype(bf16),
            "ones": ones, "onesbf": np.ones((128, 32), dtype=bf16),
            "ones32": np.full((32, 128), 1.0 / 32.0, dtype=np.float32),
            "identbf": np.eye(128, dtype=bf16),
        })
    return in_maps


def assemble_output(results):
    out = np.empty((B, S, HID), dtype=np.float32)
    for c in range(NCORES):
        r = results[c]["outT"]                 # [HID, 2*SPC]
        out[0, c * SPC:(c + 1) * SPC, :] = r[:, 0:SPC].T
        out[1, c * SPC:(c + 1) * SPC, :] = r[:, SPC:2 * SPC].T
    return out


def kernel(hidden_states, q_w, kv_a_w, kv_b_w, o_w, kv_norm_w):
    from concourse import bass_utils

    if "nc" not in _CACHE:
        _CACHE["nc"] = build_nc()
    nc = _CACHE["nc"]
    in_maps = build_in_maps(hidden_states, q_w, kv_a_w, kv_b_w, o_w, kv_norm_w)
    res = bass_utils.run_bass_kernel_spmd(
        nc, in_maps, core_ids=list(range(NCORES)), trace=False)
    return assemble_output(res.results)
